# revision 9
# baseline (speedup 1.0000x reference)
"""Trainium2 Bass kernel for nn_Attention_74586402062900.

Bahdanau-attention + GRU decoder, 32 sequential steps.
Data-parallel over batch: B=256 -> 32 per core on 8 NeuronCores.

Per-core layout strategy (all matmul operands bf16, fp32 PSUM accumulate):
  fp      [h(4x128), (t,b)]   attention projection, SBUF-resident
  feats   [t(128), (b,c)]     context rhs
  feats   [c(4x128), (t,b)]   precompute rhs (slot shared with tanh output)
  score   pair-packed matmuls -> e^T [pair, (j,t)] PSUM rows
  softmax exp (no max-sub; |e| <= sum|w| ~ 18, safe in fp32)
  ctx     masked-alpha-diagonal matmuls -> ctx^T pairs -> PE-transpose -> [c,b]
  GRU     weights-moving matmuls, gates in [b, g] orientation
"""

import sys
import os
import numpy as np

for _p in ("/opt/trn_rl_repo", "/root/.axon_site/_ro/trn_rl_repo"):
    if os.path.isdir(_p) and _p not in sys.path:
        sys.path.insert(0, _p)

import ml_dtypes

import concourse.bass as bass
import concourse.tile as tile
import concourse.mybir as mybir
from concourse.bass_utils import run_bass_kernel_spmd

bf16 = mybir.dt.bfloat16
f32 = mybir.dt.float32
AF = mybir.ActivationFunctionType
ALU = mybir.AluOpType

# problem constants (full), hardcoded per the harness contract
NT, NB, NC, NH, NCLS, NSTEPS = 128, 256, 512, 512, 37, 32
NCORES = 8
BL = NB // NCORES       # 32 batch per core
HC = NH // 128          # 4 h-chunks
CC = NC // 128          # 4 c-chunks
NPAIR = BL // 2         # 16
G = 3 * NH              # 1536
TB = NT * BL            # 4096 (t,b) columns


def _split_waits(nc, maxw=1):
    """Walrus on this toolchain rejects >1 sync wait per instruction; move
    excess waits onto preceding same-engine NOPs."""
    engines = nc.engines
    for blk in nc.m.functions[0].blocks:
        insts = list(blk.instructions)
        need_fix = set()
        for i, inst in enumerate(insts):
            si = inst.sync_info
            if si is not None and si.on_wait is not None and len(si.on_wait) > maxw:
                need_fix.add(i)
        if not need_fix:
            continue
        new_list = []
        for i, inst in enumerate(insts):
            si = inst.sync_info
            if i in need_fix:
                waits = list(si.on_wait)
                keep, extra = waits[:maxw], waits[maxw:]
                for j in range(0, len(extra), maxw):
                    chunk = extra[j:j + maxw]
                    nop_inst = engines[inst.engine].nop(
                        hint="waitsplit", nofuse=True).ins
                    nsi = nop_inst.sync_info
                    if nsi is None:
                        nop_inst.sync_info = mybir.SyncInfo(
                            on_wait=list(chunk), on_update=[])
                    else:
                        nsi.on_wait = list(chunk)
                        nop_inst.sync_info = nsi
                    new_list.append(nop_inst)
                si.on_wait = keep
                inst.sync_info = si
            new_list.append(inst)
        created = {x.name for x in new_list} - {x.name for x in insts}
        for b2 in nc.m.functions[0].blocks:
            if b2.name == blk.name:
                b2.instructions = new_list
            else:
                b2.instructions = [x for x in b2.instructions
                                   if x.name not in created]


def build_program(S=NSTEPS):
    nc = bass.Bass()
    dt_in = [
        ("f_cb", (128, CC * TB)),        # [c-chunk part, cc*4096 + t*32+b]
        ("f_tb", (128, BL * NC)),        # [t part, b*512 + c]
        ("wi2h", (128, HC * CC * 128)),  # lhsT tiles (hc,cc) -> col block
        ("wh2h", (128, HC * HC * 128)),  # hp lhsT tiles (oc,kc)
        ("bh2h", (1, NH)),
        ("ones", (1, 128)),
        ("wsc", (128, HC * NPAIR * 8)),  # (hc,p) tiles [128,8], w at col p%8
        ("wih", (128, CC * G)),          # gi rhs chunks
        ("whh", (128, HC * G)),          # gh rhs chunks
        ("bg_rz", (1, 2 * NH)),          # b_ih+b_hh for r,z
        ("bg_in", (1, NH)),              # b_ih n-slice
        ("bg_hn", (1, NH)),              # b_hh n-slice
        ("wgen", (128, HC * NCLS)),
        ("bgen", (1, NCLS)),
        ("iden", (128, 128)),
    ]
    dr = {}
    for name, shape in dt_in:
        dr[name] = nc.dram_tensor(name, shape, bf16, kind="ExternalInput")
    probs_d = nc.dram_tensor("probs", (BL, S * NCLS), f32, kind="ExternalOutput")

    with tile.TileContext(nc) as tc:
        with tc.tile_pool(name="const", bufs=1) as cpool, \
             tc.tile_pool(name="big", bufs=1) as bigp, \
             tc.tile_pool(name="state", bufs=1) as stp, \
             tc.tile_pool(name="work", bufs=1) as wkp, \
             tc.tile_pool(name="ps", bufs=1, space="PSUM") as psp:

            cst = {}
            for name, shape in dt_in:
                cst[name] = cpool.tile(list(shape), bf16, tag=name, name=name)
            # shared-slot big tiles: f_cb (precompute only) then tm reuses it
            f_cb = bigp.tile([128, CC * TB], bf16, tag="bigslot")
            for name, _ in dt_in:
                if name != "f_cb":
                    nc.sync.dma_start(cst[name][:], dr[name][:])
            nc.sync.dma_start(f_cb[:], dr["f_cb"][:])
            f_tb, wi2h, wh2h = cst["f_tb"], cst["wi2h"], cst["wh2h"]
            bh2h, ones, wsc = cst["bh2h"], cst["ones"], cst["wsc"]
            wih, whh = cst["wih"], cst["whh"]
            bg_rz, bg_in, bg_hn = cst["bg_rz"], cst["bg_in"], cst["bg_hn"]
            wgen, bgen, iden = cst["wgen"], cst["bgen"], cst["iden"]

            fp = stp.tile([128, HC * TB], bf16)     # attention projection
            h_cb = stp.tile([128, HC * BL], bf16)   # h, [h-chunk, kc*32+b]
            hT = stp.tile([BL, NH], bf16)           # h, [b, h]
            hp_sb = stp.tile([128, HC * BL], bf16)
            amask = [stp.tile([128, NPAIR * NPAIR], bf16, tag=f"am{j}", name=f"am{j}")
                     for j in range(2)]
            probs_sb = stp.tile([BL, S * NCLS], f32)

            nc.vector.memset(h_cb[:], 0)
            nc.vector.memset(hT[:], 0)
            nc.vector.memset(amask[0][:], 0)
            nc.vector.memset(amask[1][:], 0)

            # ---- precompute fp = feats @ W_i2h.T ----
            NSEG = TB // 512  # 8
            for hc in range(HC):
                for seg in range(NSEG):
                    fps = psp.tile([128, 512], f32, tag="ps_a")
                    for cc in range(CC):
                        nc.tensor.matmul(
                            fps[:],
                            wi2h[:, bass.ts(hc * CC + cc, 128)],
                            f_cb[:, cc * TB + seg * 512:cc * TB + (seg + 1) * 512],
                            start=(cc == 0), stop=(cc == CC - 1))
                    nc.vector.tensor_copy(
                        fp[:, hc * TB + seg * 512:hc * TB + (seg + 1) * 512],
                        fps[:])

            tm = bigp.tile([128, HC * TB], bf16, tag="bigslot")  # tanh out

            # ---- step loop ----
            for s in range(S):
                # hp = W_h2h @ h + b_h2h   -> [h-chunk part, oc*32+b]
                hp_ps = psp.tile([128, HC * BL], f32, tag="ps_a")
                for oc in range(HC):
                    for kc in range(HC):
                        nc.tensor.matmul(
                            hp_ps[:, bass.ts(oc, BL)],
                            wh2h[:, bass.ts(oc * HC + kc, 128)],
                            h_cb[:, bass.ts(kc, BL)],
                            start=(kc == 0), stop=False)
                    nc.tensor.matmul(
                        hp_ps[:, bass.ts(oc, BL)],
                        bh2h[:, bass.ts(oc, 128)],
                        ones[:, 0:BL],
                        start=False, stop=True)
                    nc.vector.tensor_copy(
                        hp_sb[:, bass.ts(oc, BL)], hp_ps[:, bass.ts(oc, BL)])

                # attention: b-half-major so ctx of half 0 overlaps tanh of half 1
                ctx_ps = [psp.tile([NPAIR, 512], f32, tag=f"ps_cg{j}", name=f"ctx{j}")
                          for j in range(2)]
                for g in range(2):
                    e_ps = psp.tile([8, 256], f32, tag=f"ps_e{g}", name=f"eps{g}")
                    for hc in range(HC):
                        lo = hc * TB
                        fp3 = fp[:, lo:lo + TB].rearrange(
                            "p (t b) -> p t b", b=BL)[:, :, g * 16:(g + 1) * 16]
                        tm3 = tm[:, lo:lo + TB].rearrange(
                            "p (t b) -> p t b", b=BL)[:, :, g * 16:(g + 1) * 16]
                        hpb = hp_sb[:, hc * BL + g * 16:hc * BL + (g + 1) * 16] \
                            .unsqueeze(1).broadcast_to([128, NT, 16])
                        nc.vector.tensor_add(tm3, fp3, hpb)
                        nc.scalar.activation(tm3, tm3, AF.Tanh)
                        tmh = tm[:, lo:lo + TB].rearrange(
                            "p (t b) -> p t b", b=BL)
                        for lp in range(8):
                            p = g * 8 + lp
                            rhs = tmh[:, :, 2 * p:2 * p + 2] \
                                .rearrange("p t j -> p j t")
                            nc.tensor.matmul(
                                e_ps[:],
                                wsc[:, bass.ts(hc * NPAIR + p, 8)],
                                rhs,
                                start=(hc == 0 and lp == 0),
                                stop=(hc == HC - 1 and lp == 7))

                    aN = wkp.tile([8, 256], bf16, tag=f"aN{g}", name=f"aN{g}")
                    Ssum = wkp.tile([8, 2], f32, tag=f"Ss{g}", name=f"Ss{g}")
                    Srec = wkp.tile([8, 2], f32, tag=f"Sr{g}", name=f"Sr{g}")
                    nc.scalar.activation(aN[:], e_ps[:], AF.Exp)
                    nc.vector.tensor_reduce(
                        Ssum[:],
                        aN[:].rearrange("p (j t) -> p j t", j=2),
                        axis=mybir.AxisListType.X, op=ALU.add)
                    nc.vector.reciprocal(Srec[:], Ssum[:])
                    aNv = aN[:].rearrange("p (j t) -> p j t", j=2)
                    srb = Srec[:].unsqueeze(2).broadcast_to([8, 2, NT])
                    nc.vector.tensor_tensor(aNv, aNv, srb, ALU.mult)
                    # transpose halves into diagonal masked alpha tiles
                    for j in range(2):
                        atp = psp.tile([128, 8], bf16, tag=f"ps_e{g}",
                                       name=f"atp{g}{j}")
                        nc.tensor.transpose(
                            atp[:], aN[:, bass.ts(j, NT)], iden[0:8, 0:8])
                        nc.vector.tensor_copy(
                            amask[j][:, 136 * g:256:17][:, 0:8], atp[:])

                    # ctx for this half's 16 b's
                    for bl in range(16):
                        b = g * 16 + bl
                        j, p = b % 2, b // 2
                        nc.tensor.matmul(
                            ctx_ps[j][:],
                            amask[j][:, bass.ts(p, NPAIR)],
                            f_tb[:, b * NC:(b + 1) * NC],
                            start=(b < 2), stop=(b >= BL - 2))

                # ctx^T pairs -> [c,b] via PE transposes
                ctxT = wkp.tile([NPAIR, 2 * NC], bf16, tag="ctxT")
                nc.vector.tensor_copy(ctxT[:, 0:NC], ctx_ps[0][:])
                nc.vector.tensor_copy(ctxT[:, NC:2 * NC], ctx_ps[1][:])
                ctx_cb = wkp.tile([128, CC * BL], bf16, tag="ctxcb")
                for cc in range(CC):
                    for j in range(2):
                        ctp = psp.tile([128, NPAIR], bf16, tag=f"ps_e{j}")
                        nc.tensor.transpose(
                            ctp[:],
                            ctxT[:, j * NC + cc * 128:j * NC + (cc + 1) * 128],
                            iden[0:NPAIR, 0:NPAIR])
                        nc.vector.tensor_copy(
                            ctx_cb[:, cc * BL + j:(cc + 1) * BL:2], ctp[:])

                # GRU gates: rz joint (gi+gh+bias), n separate
                rz_ps = [psp.tile([BL, 512], f32, tag=f"ps_rz{j}", name=f"rz{j}")
                         for j in range(2)]
                gin_ps = psp.tile([BL, NH], f32, tag="ps_cg0")
                ghn_ps = psp.tile([BL, NH], f32, tag="ps_cg1")
                for seg in range(2):
                    for kc in range(HC):
                        nc.tensor.matmul(
                            rz_ps[seg][:], h_cb[:, bass.ts(kc, BL)],
                            whh[:, kc * G + seg * 512:kc * G + (seg + 1) * 512],
                            start=(kc == 0), stop=False)
                    for kc in range(CC):
                        nc.tensor.matmul(
                            rz_ps[seg][:], ctx_cb[:, bass.ts(kc, BL)],
                            wih[:, kc * G + seg * 512:kc * G + (seg + 1) * 512],
                            start=False, stop=False)
                    nc.tensor.matmul(
                        rz_ps[seg][:], ones[:, 0:BL],
                        bg_rz[:, bass.ts(seg, 512)],
                        start=False, stop=True)
                for kc in range(HC):
                    nc.tensor.matmul(
                        ghn_ps[:], h_cb[:, bass.ts(kc, BL)],
                        whh[:, kc * G + 1024:kc * G + 1536],
                        start=(kc == 0), stop=False)
                nc.tensor.matmul(
                    ghn_ps[:], ones[:, 0:BL], bg_hn[:],
                    start=False, stop=True)
                for kc in range(CC):
                    nc.tensor.matmul(
                        gin_ps[:], ctx_cb[:, bass.ts(kc, BL)],
                        wih[:, kc * G + 1024:kc * G + 1536],
                        start=(kc == 0), stop=False)
                nc.tensor.matmul(
                    gin_ps[:], ones[:, 0:BL], bg_in[:],
                    start=False, stop=True)

                # gate elementwise: r=sig(rz0), z=sig(rz1)
                # n = tanh(gin + (0.5*tanh(0.5*rz0)+0.5) * ghn)
                # h' = n + (0.5*tanh(0.5*rz1)+0.5) * (h - n)
                r_t = wkp.tile([BL, NH], bf16, tag="rt")
                z_t = wkp.tile([BL, NH], bf16, tag="zt")
                nc.scalar.activation(r_t[:], rz_ps[0][:], AF.Tanh, scale=0.5)
                nc.scalar.activation(z_t[:], rz_ps[1][:], AF.Tanh, scale=0.5)
                rs = wkp.tile([BL, NH], bf16, tag="rs")
                nc.vector.tensor_scalar(
                    rs[:], r_t[:], 0.5, 0.5, ALU.mult, ALU.add)
                zs = wkp.tile([BL, NH], bf16, tag="zs")
                nc.vector.tensor_scalar(
                    zs[:], z_t[:], 0.5, 0.5, ALU.mult, ALU.add)
                ghn_sb = wkp.tile([BL, NH], bf16, tag="ghnsb")
                nc.vector.tensor_copy(ghn_sb[:], ghn_ps[:])
                rh = wkp.tile([BL, NH], bf16, tag="rh")
                nc.vector.tensor_mul(rh[:], rs[:], ghn_sb[:])
                nin = wkp.tile([BL, NH], bf16, tag="nin")
                nc.vector.tensor_add(nin[:], rh[:], gin_ps[:])
                n_t = wkp.tile([BL, NH], bf16, tag="nt")
                nc.scalar.activation(n_t[:], nin[:], AF.Tanh)
                d = wkp.tile([BL, NH], bf16, tag="d")
                nc.vector.tensor_sub(d[:], hT[:], n_t[:])
                zd = wkp.tile([BL, NH], bf16, tag="zd")
                nc.vector.tensor_mul(zd[:], zs[:], d[:])
                nc.vector.tensor_add(hT[:], n_t[:], zd[:])

                # h^T -> h_cb via PE transposes
                for kc in range(HC):
                    htp = psp.tile([128, BL], bf16, tag=f"ps_e{kc % 2}")
                    nc.tensor.transpose(
                        htp[:], hT[:, bass.ts(kc, 128)], iden[0:BL, 0:BL])
                    nc.vector.tensor_copy(h_cb[:, bass.ts(kc, BL)], htp[:])

                # gen: probs_s = h @ W_gen.T + b_gen
                gen_ps = psp.tile([BL, NCLS], f32, tag="ps_a")
                for kc in range(HC):
                    nc.tensor.matmul(
                        gen_ps[:], h_cb[:, bass.ts(kc, BL)],
                        wgen[:, bass.ts(kc, NCLS)],
                        start=(kc == 0), stop=False)
                nc.tensor.matmul(
                    gen_ps[:], ones[:, 0:BL], bgen[:],
                    start=False, stop=True)
                nc.vector.tensor_copy(
                    probs_sb[:, bass.ts(s, NCLS)], gen_ps[:])

            nc.sync.dma_start(probs_d[:], probs_sb[:])

    _split_waits(nc, maxw=1)
    return nc



_BF = ml_dtypes.bfloat16


def _prep_host(inputs):
    feats = np.asarray(inputs["feats"], np.float32)           # [T,B,C]
    W_i2h = np.asarray(inputs["W_i2h"], np.float32)
    W_h2h = np.asarray(inputs["W_h2h"], np.float32)
    b_h2h = np.asarray(inputs["b_h2h"], np.float32)
    w_score = np.asarray(inputs["w_score"], np.float32)
    W_ih = np.asarray(inputs["W_ih"], np.float32)
    b_ih = np.asarray(inputs["b_ih"], np.float32)
    W_hh = np.asarray(inputs["W_hh"], np.float32)
    b_hh = np.asarray(inputs["b_hh"], np.float32)
    W_gen = np.asarray(inputs["W_gen"], np.float32)
    b_gen = np.asarray(inputs["b_gen"], np.float32)

    # weight layouts (shared across cores)
    wi2h = np.zeros((128, HC * CC * 128), np.float32)
    WT = W_i2h.T  # [C, H]
    for hc in range(HC):
        for cc in range(CC):
            wi2h[:, (hc * CC + cc) * 128:(hc * CC + cc + 1) * 128] = \
                WT[cc * 128:(cc + 1) * 128, hc * 128:(hc + 1) * 128]
    wh2h = np.zeros((128, HC * HC * 128), np.float32)
    WT = W_h2h.T  # [Hin, Hout]
    for oc in range(HC):
        for kc in range(HC):
            wh2h[:, (oc * HC + kc) * 128:(oc * HC + kc + 1) * 128] = \
                WT[kc * 128:(kc + 1) * 128, oc * 128:(oc + 1) * 128]
    wsc = np.zeros((128, HC * NPAIR * 8), np.float32)
    for hc in range(HC):
        for p in range(NPAIR):
            wsc[:, (hc * NPAIR + p) * 8 + (p % 8)] = \
                w_score[hc * 128:(hc + 1) * 128]
    wih = np.concatenate([W_ih.T[kc * 128:(kc + 1) * 128, :]
                          for kc in range(CC)], axis=1)  # [128, CC*G]
    whh = np.concatenate([W_hh.T[kc * 128:(kc + 1) * 128, :]
                          for kc in range(HC)], axis=1)
    bg = b_ih + b_hh
    wgen = np.concatenate([W_gen.T[kc * 128:(kc + 1) * 128, :]
                           for kc in range(HC)], axis=1)  # [128, HC*37]

    const = {
        "wi2h": wi2h, "wh2h": wh2h,
        "bh2h": b_h2h.reshape(1, NH),
        "ones": np.ones((1, 128), np.float32),
        "wsc": wsc, "wih": wih, "whh": whh,
        "bg_rz": bg[:2 * NH].reshape(1, -1),
        "bg_in": b_ih[2 * NH:].reshape(1, -1),
        "bg_hn": b_hh[2 * NH:].reshape(1, -1),
        "wgen": wgen, "bgen": b_gen.reshape(1, NCLS),
        "iden": np.eye(128, dtype=np.float32),
    }
    const = {k: np.ascontiguousarray(v, dtype=None).astype(_BF)
             for k, v in const.items()}

    in_maps = []
    for core in range(NCORES):
        fsh = feats[:, core * BL:(core + 1) * BL, :]          # [T, BL, C]
        f_cb = fsh.transpose(2, 0, 1).reshape(CC, 128, TB) \
            .transpose(1, 0, 2).reshape(128, CC * TB).astype(_BF)
        f_tb = fsh.transpose(0, 1, 2).reshape(NT, BL * NC).astype(_BF)
        m = dict(const)
        m["f_cb"] = np.ascontiguousarray(f_cb)
        m["f_tb"] = np.ascontiguousarray(f_tb)
        in_maps.append(m)
    return in_maps


_CACHE = {}


def kernel(**inputs):
    num_steps = int(np.asarray(inputs.get("num_steps", NSTEPS)))
    assert num_steps == NSTEPS, f"kernel hardcoded for {NSTEPS} steps"
    if "nc" not in _CACHE:
        _CACHE["nc"] = build_program(NSTEPS)
    nc = _CACHE["nc"]
    in_maps = _prep_host(inputs)
    res = run_bass_kernel_spmd(nc, in_maps, core_ids=list(range(NCORES)))
    outs = []
    for core in range(NCORES):
        p = res.results[core]["probs"]          # [BL, S*NCLS]
        outs.append(p.reshape(BL, NSTEPS, NCLS).reshape(BL * NSTEPS, NCLS))
    return np.concatenate(outs, axis=0).astype(np.float32)
